# revision 27
# baseline (speedup 1.0000x reference)
"""AnchorTargetLayer max-IoU kernel for 8 TRN2 NeuronCores (v7.3).

max_iou[b, n] = max_g IoU(anchor_n, gt_box[b, g]);
anchors [100000, 4] f32, gt_boxes [4, 64, 4] f32 -> out [4, 100000] f32.

Sharding: anchors split 8 ways (12544/core incl pad), gt replicated, no
collectives.  HW exec ~132-134us vs the v5 baseline's 167.6us.

Layout (v7, vs v5's anchors-on-partitions): (batch, gt) pairs live on
PARTITIONS - p = b*32 + gh, gl in {0,1} on the free dim (g = gh*2+gl) -
and ANCHORS live on the FREE dim, broadcast to all 128 partitions by a
partition-stride-0 DMA (fully overlapped, ~380 GB/s).  This replaces
v5's 392 narrow 256-wide tensor_scalar ops (~360ns each, fixed-overhead
bound) with a handful of wide ops per chunk.

Chunks of <=3136 anchors ([1024, 3136, 2112, 3136, 2112, 1024]: small
head chunk starts the pipeline early; the first y-relu chunk (2) is
small and follows a big clip-form chunk so the ACT engine's relus for
it land well before DVE needs them (otherwise a DMA-timing race makes
runs bimodal 132/143us); small tail chunk shortens the drain; chunk 0's
broadcast is split x-rows-first so clips-x start ~3us earlier).  Per chunk, all fp16:
  x spans:  CHL[gl] = clamp(ax1/ax2 rows; gx1[p], gx2[p])  2 TS 2-op @4x
            Ix = CHL[hi] - CHL[lo]  (>= 0 by clipping)     1 wide TT
  y spans:  chunks 0-1 (clip form, no ACT dependency at pipeline head):
              same as x on DVE;
            chunks 2+ (relu form, offloaded to the idle ACT engine):
              ACT: r1y = relu(-ay2 + gy2[p]), r2y = relu(ay1 - gy1[p])
              DVE: SY = (r1y + r2y)*(-1) + gheight[p]   (TT + TS @4x)
  int:      INT = Ix * IY                                  1 TT
  w:        W = INT * RS;  RS = 1/(aarea + garea[p]) on ACT (Reciprocal
            with per-partition bias: no separate area-sum pass)
  gl max:   R0 = max(W[gl0], W[gl1])                       1 TT
  gh max:   stream_transpose 32x32 (b-major partitions put gh innermost
            per anchor subblock), then a 5-step pairwise TT-max tree.
Fixups per chunk (overlapped): ACT R1 = 1/(1 - v); DVE STT
MIOU = max(v, 0) * R1  (w = int/(areaA+areaG) is monotone in iou, so
iou = v/(1-v) after the max; the max(.,0) guards relu-form negatives).
Per-chunk output DMA drains while later chunks compute.

Notes: w <= 1/2 so 1-v >= 1/2 (no cancellation); scalar_tensor_tensor
runs at 1x but only on the tiny [P, fc/32] fixup.  SBUF: the CHL slot
block is reused for SY/INT/W/R0/ST/tree scratch (chain is serial on
DVE); ABC, RS, RRY are double-buffered across chunks.  Engine balance:
DVE ~110us busy, ACT ~60us, head ~18us is DMA-latency bound.

Learned constraints (measured/compile-tested this session): TT needs
equal base partitions (no partition-offset tree); TT reads at most one
PSUM operand; compute ops cannot write at a partition offset; custom
DVE ops and gpsimd partition_all_reduce fail walrus codegen ("ISA wrong
length"); Pool TT is rejected ("engine check failed"); stream_transpose
free dims of 800 corrupt downstream 128-col groups (1024/1568/2112/3136
are clean); DMA-transpose tiles serialize at ~1.24us/tile on one queue.
"""

import os
import sys

import numpy as np

sys.path.insert(0, "/opt/trn_rl_repo")

import concourse.bass as bass
import concourse.mybir as mybir
from concourse.bass_utils import run_bass_kernel_spmd

N_ANCHORS = 100000
BATCH = 4
N_GT = 64
N_CORES = 8

P = 128
FC = 3136                   # max anchors per chunk
# small head chunks for fast pipeline start; small tail chunk shortens drain
CHUNKS = [1024, 3136, 2112, 3136, 2112, 1024]
NCH = len(CHUNKS)
N_LOC = sum(CHUNKS)         # 12544
N_PAD = N_LOC * N_CORES     # 100352
NB = 2                      # ABC / RS double-buffer depth

F32 = mybir.dt.float32
F16 = mybir.dt.float16
COORD_SCALE = 1.0 / 16.0

LAST_EXEC_NS = None


def _ensure_axon_ntff_hook():
    try:
        import antenv.axon_hooks  # noqa: F401

        return
    except ImportError:
        pass
    import contextlib
    import ctypes
    import types

    import antenv

    m = types.ModuleType("antenv.axon_hooks")
    m._hook = None

    def set_axon_ntff_profile_hook(h):
        m._hook = h

    def get_axon_ntff_profile_hook():
        return m._hook

    m.set_axon_ntff_profile_hook = set_axon_ntff_profile_hook
    m.get_axon_ntff_profile_hook = get_axon_ntff_profile_hook
    sys.modules["antenv.axon_hooks"] = m
    antenv.axon_hooks = m

    so_path = os.environ.get("PJRT_LIBRARY_PATH", "/opt/axon/libaxon_pjrt.so")
    try:
        lib = ctypes.CDLL(so_path)
    except OSError:
        return
    if not hasattr(lib, "axon_start_nrt_profile"):
        return
    lib.axon_start_nrt_profile.argtypes = [
        ctypes.POINTER(ctypes.c_int64),
        ctypes.c_size_t,
    ]
    lib.axon_start_nrt_profile.restype = ctypes.c_int64
    lib.axon_stop_nrt_profile.argtypes = [ctypes.c_char_p]
    lib.axon_stop_nrt_profile.restype = ctypes.c_int64

    @contextlib.contextmanager
    def _hook(output_dir, device_ids):
        import jax

        jax.devices()
        if device_ids:
            ids = (ctypes.c_int64 * len(device_ids))(*device_ids)
            rc = lib.axon_start_nrt_profile(ids, len(device_ids))
        else:
            rc = lib.axon_start_nrt_profile(None, 0)
        if rc != 0:
            raise RuntimeError(f"axon_start_nrt_profile rc={rc}")
        try:
            yield
        finally:
            n = lib.axon_stop_nrt_profile(str(output_dir).encode())
            if n < 0:
                raise RuntimeError(f"axon_stop_nrt_profile rc={n}")

    set_axon_ntff_profile_hook(_hook)


def _patch_upload_artifacts():
    import concourse.bass_utils as bu

    if getattr(bu.upload_artifacts, "_safe", False):
        return
    orig = bu.upload_artifacts

    def safe(tmpdir):
        try:
            return orig(tmpdir)
        except Exception:
            return tmpdir

    safe._safe = True
    bu.upload_artifacts = safe


def _act_recip(scalar_eng, nc, out_ap, in_ap, bias=0.0, scale=1.0):
    """Emit Activation(Reciprocal); bias may be a [P,1] AP or a float.
    (The nc.scalar.activation wrapper rejects Reciprocal.)"""
    ins = [scalar_eng.lower_ap(in_ap)]
    if isinstance(bias, (int, float)):
        ins.append(mybir.ImmediateValue(dtype=F32, value=float(bias)))
    else:
        ins.append(scalar_eng.lower_ap(bias))
    ins.append(mybir.ImmediateValue(dtype=F32, value=float(scale)))
    ins.append(mybir.ImmediateValue(dtype=F32, value=0.0))
    return scalar_eng.add_instruction(
        mybir.InstActivation(
            name=nc.get_next_instruction_name(),
            func=mybir.ActivationFunctionType.Reciprocal,
            ins=ins,
            outs=[scalar_eng.lower_ap(out_ap)],
        )
    )


def _build_graph():
    nc = bass.Bass()
    AR_ext = nc.declare_dram_parameter("arows", [5, N_LOC], F16, isOutput=False)
    GC_ext = nc.declare_dram_parameter("gcols", [P, 14], F32, isOutput=False)
    NFB = N_LOC // 32
    out_ext = nc.declare_dram_parameter("out", [P, NFB], F32, isOutput=True)

    Alu = mybir.AluOpType
    from contextlib import ExitStack

    with ExitStack() as _st:
        e = _st.enter_context

        ABC = e(nc.sbuf_tensor("ABC", [P, NB, 5, FC], F16))
        GCS = e(nc.sbuf_tensor("GCS", [P, 14], F32))
        # CHL: clip outputs [ax, gl, endp(lo,hi), FC]; its 8*FC region is
        # reused downstream: INT <- CHL[0,0], W <- CHL[0,1], R0/ST <- CHL[1,0],
        # tree scratch <- CHL[1,1] (the whole chain is serial on DVE).
        CHL = e(nc.sbuf_tensor("CHL", [P, 2, 2, 2, FC], F16))
        I = e(nc.sbuf_tensor("I", [P, 2, FC], F16))
        RRY = e(nc.sbuf_tensor("RRY", [P, NB, 2, 2, FC], F16))
        RS = e(nc.sbuf_tensor("RS", [P, NB, 2, FC], F16))
        VOUT = e(nc.sbuf_tensor("VOUT", [P, NFB], F16))
        R1 = e(nc.sbuf_tensor("R1", [P, NFB], F16))
        MIOU = e(nc.sbuf_tensor("MIOU", [P, NFB], F32))

        block = e(nc.Block())
        dma_sem = e(nc.semaphore("dma_sem"))
        dma_act = e(nc.semaphore("dma_act"))
        dve_sem = e(nc.semaphore("dve_sem"))
        miou_sem = e(nc.semaphore("miou_sem"))
        act_rs = e(nc.semaphore("act_rs"))
        act_r1 = e(nc.semaphore("act_r1"))

        def gcol(i):
            return GCS[:, i : i + 1]

        offs = [sum(CHUNKS[:i]) for i in range(NCH)]
        fbo = [o // 32 for o in offs]

        # dve_sem ticks: 2c+1 = clips(c) done, 2c+2 = tree(c) done.
        # x-rows (0:2) broadcast on the SP queue (consumed by DVE clips);
        # y+area rows (2:5) broadcast in parallel on the ACT queue
        # (consumed by the ACT reciprocals; y rows are spare for now).
        def bcast(eng, c, rows, sem):
            fc = CHUNKS[c]
            a_ap = AR_ext[rows[0] : rows[1], offs[c] : offs[c] + fc]
            a_b = bass.AP(
                tensor=a_ap.tensor, offset=a_ap.offset,
                ap=[[0, P]] + list(a_ap.ap),
            )
            return eng.dma_start(
                out=ABC[:, c % NB, rows[0] : rows[1], 0:fc], in_=a_b
            ).then_inc(sem, 16)

        @block.sync
        def _(sync):
            sync.dma_start(out=GCS[:, :], in_=GC_ext[:, :]).then_inc(dma_sem, 16)
            for c in range(NCH):
                if c >= NB:
                    sync.wait_ge(dve_sem, 2 * c - 3)
                    sync.wait_ge(act_rs, c - 1)
                if c == 0:
                    # split: x rows land first so clips-x(0) starts early
                    bcast(sync, 0, (0, 2), dma_sem)
                    bcast(sync, 0, (2, 5), dma_sem)
                else:
                    bcast(sync, c, (0, 5), dma_sem)
            # per-chunk output drains as soon as each MIOU slab is written
            for c in range(NCH):
                sync.wait_ge(miou_sem, c + 1)
                sync.dma_start(
                    out=out_ext[:, fbo[c] : fbo[c] + CHUNKS[c] // 32],
                    in_=MIOU[:, fbo[c] : fbo[c] + CHUNKS[c] // 32],
                ).then_inc(dma_sem, 16)
            sync.wait_ge(dma_sem, 16 * (2 * NCH + 2))

        @block.vector
        def _(vector):
            for c in range(NCH):
                cb = c % NB
                fc = CHUNKS[c]
                fb = fc // 32
                # 4 clip TS 2-op @4x: CHL[ax,gl] = (a_rows max g1) min g2
                # x rows arrive on the SP queue, y rows on the ACT queue
                vector.wait_ge(dma_sem, 16 * (2 + c) if c == 0 else 16 * (3 + c))
                yrelu = c >= 2
                # x clips always; y clips only for head (clip-form) chunks
                for ax in range(2 if not yrelu else 1):
                    if c == 0 and ax == 1:
                        vector.wait_ge(dma_sem, 48)  # y/area rows of chunk 0
                    rows = ABC[:, cb, 2 * ax : 2 * ax + 2, 0:fc]  # [P, 2, fc]
                    for gl in range(2):
                        clip_ins = vector.tensor_scalar(
                            out=CHL[:, ax, gl, :, 0:fc], in0=rows,
                            scalar1=gcol(4 * gl + 2 * ax + 0),
                            scalar2=gcol(4 * gl + 2 * ax + 1),
                            op0=Alu.max, op1=Alu.min,
                        )
                clip_ins.then_inc(dve_sem, 1)  # tick 2c+1
                # fixup of the previous chunk rides here (ACT slack);
                # relu via STT guards y-relu chunks' possibly-negative v
                if c >= 1:
                    vector.wait_ge(act_r1, c)
                    pc = c - 1
                    vector.scalar_tensor_tensor(
                        out=MIOU[:, fbo[pc] : fbo[pc] + CHUNKS[pc] // 32],
                        in0=VOUT[:, fbo[pc] : fbo[pc] + CHUNKS[pc] // 32],
                        scalar=0.0,
                        in1=R1[:, fbo[pc] : fbo[pc] + CHUNKS[pc] // 32],
                        op0=Alu.max, op1=Alu.mult,
                    ).then_inc(miou_sem, 1)
                # Ix = hi - lo  [P, 2gl, fc]
                vector.tensor_tensor(
                    out=I[:, :, 0:fc],
                    in0=CHL[:, 0, :, 1, 0:fc],
                    in1=CHL[:, 0, :, 0, 0:fc],
                    op=Alu.subtract,
                )
                if yrelu:
                    # SY = gh - r1y - r2y  (ACT supplied RRY)
                    vector.wait_ge(act_rs, c + 1)
                    vector.tensor_tensor(
                        out=CHL[:, 1, :, 0, 0:fc],
                        in0=RRY[:, cb, :, 0, 0:fc], in1=RRY[:, cb, :, 1, 0:fc],
                        op=Alu.add,
                    )
                    for gl in range(2):
                        vector.tensor_scalar(
                            out=CHL[:, 1, gl, 1, 0:fc],
                            in0=CHL[:, 1, gl, 0, 0:fc],
                            scalar1=-1.0, scalar2=gcol(12 + gl),
                            op0=Alu.mult, op1=Alu.add,
                        )
                    iy = CHL[:, 1, :, 1, 0:fc]
                else:
                    # Iy via clip-form: hi - lo from the y clips
                    vector.tensor_tensor(
                        out=CHL[:, 0, :, 0, 0:fc],
                        in0=CHL[:, 1, :, 1, 0:fc],
                        in1=CHL[:, 1, :, 0, 0:fc],
                        op=Alu.subtract,
                    )
                    iy = CHL[:, 0, :, 0, 0:fc]
                # INT = Ix * Iy  [P, 2gl, fc]
                int_out = CHL[:, 0, :, 1, 0:fc] if not yrelu else CHL[:, 0, :, 0, 0:fc]
                vector.tensor_tensor(
                    out=int_out, in0=I[:, :, 0:fc], in1=iy, op=Alu.mult,
                )
                # W = INT * RS
                if not yrelu:
                    vector.wait_ge(act_rs, c + 1)
                w_out = CHL[:, 1, :, 0, 0:fc] if not yrelu else CHL[:, 0, :, 1, 0:fc]
                vector.tensor_tensor(
                    out=w_out, in0=int_out, in1=RS[:, cb, :, 0:fc], op=Alu.mult,
                )
                # R0 = max over gl
                r0_out = CHL[:, 1, 0, 1, 0:fc] if not yrelu else CHL[:, 1, 0, 0, 0:fc]
                vector.tensor_tensor(
                    out=r0_out, in0=w_out[:, 0, :], in1=w_out[:, 1, :],
                    op=Alu.max,
                )
                # 32x32 block transpose puts gh innermost per anchor subblock
                st_out = CHL[:, 1, 1, 1, 0:fc] if not yrelu else CHL[:, 1, 1, 0, 0:fc]
                vector.transpose(out=st_out, in_=r0_out)
                # pairwise-max tree over gh: 32 -> 1
                TREE = (CHL[:, 1, 1, 0, :] if not yrelu else CHL[:, 1, 0, 1, :])
                src = st_out.rearrange("p (fb g) -> p fb g", g=32)
                w = 16
                off = 0
                while w >= 2:
                    dst = TREE[:, off : off + fb * w].rearrange(
                        "p (fb g) -> p fb g", g=w
                    )
                    vector.tensor_tensor(
                        out=dst, in0=src[:, :, 0:w], in1=src[:, :, w : 2 * w],
                        op=Alu.max,
                    )
                    src = dst
                    off += fb * w
                    w //= 2
                vector.tensor_tensor(
                    out=VOUT[:, fbo[c] : fbo[c] + fb].rearrange(
                        "p (fb g) -> p fb g", g=1
                    ),
                    in0=src[:, :, 0:1], in1=src[:, :, 1:2], op=Alu.max,
                ).then_inc(dve_sem, 1)  # tick 2c+2
            # last chunk fixup
            c = NCH - 1
            vector.wait_ge(act_r1, NCH)
            vector.scalar_tensor_tensor(
                out=MIOU[:, fbo[c] : fbo[c] + CHUNKS[c] // 32],
                in0=VOUT[:, fbo[c] : fbo[c] + CHUNKS[c] // 32],
                scalar=0.0,
                in1=R1[:, fbo[c] : fbo[c] + CHUNKS[c] // 32],
                op0=Alu.max, op1=Alu.mult,
            ).then_inc(miou_sem, 1)

        @block.scalar
        def _(scalar):
            for c in range(NCH):
                cb = c % NB
                fc = CHUNKS[c]
                scalar.wait_ge(dma_sem, 48 if c == 0 else 16 * (3 + c))
                if c >= 2:
                    if c >= 4:
                        # RRY[cb] reuse: r12y(c-2) done once clips(c-1) ran
                        scalar.wait_ge(dve_sem, 2 * c - 1)
                    # y-relu form: r1y = relu(-y2 + gy2), r2y = relu(y1 - gy1)
                    for gl in range(2):
                        scalar.activation(
                            out=RRY[:, cb, gl, 0, 0:fc], in_=ABC[:, cb, 3, 0:fc],
                            func=mybir.ActivationFunctionType.Relu,
                            bias=gcol(4 * gl + 3), scale=-1.0,
                        )
                        scalar.activation(
                            out=RRY[:, cb, gl, 1, 0:fc], in_=ABC[:, cb, 2, 0:fc],
                            func=mybir.ActivationFunctionType.Relu,
                            bias=gcol(10 + gl), scale=1.0,
                        )
                if c >= NB:
                    # W(c-2) has read RS[cb]: implied by clips(c-1) done
                    scalar.wait_ge(dve_sem, 2 * c - 1)
                area = ABC[:, cb, 4, 0:fc]
                _act_recip(scalar, nc, RS[:, cb, 0, 0:fc], area, bias=gcol(8))
                _act_recip(
                    scalar, nc, RS[:, cb, 1, 0:fc], area, bias=gcol(9)
                ).then_inc(act_rs, 1)
                # R1 fixup for an earlier finished chunk: interleave to avoid
                # stalling the recips; chunk c-1's tree is done by now or soon
                if c >= 1:
                    pc = c - 1
                    scalar.wait_ge(dve_sem, 2 * pc + 2)
                    _act_recip(
                        scalar, nc,
                        R1[:, fbo[pc] : fbo[pc] + CHUNKS[pc] // 32],
                        VOUT[:, fbo[pc] : fbo[pc] + CHUNKS[pc] // 32],
                        bias=1.0, scale=-1.0,
                    ).then_inc(act_r1, 1)
            c = NCH - 1
            scalar.wait_ge(dve_sem, 2 * c + 2)
            _act_recip(
                scalar, nc, R1[:, fbo[c] : fbo[c] + CHUNKS[c] // 32],
                VOUT[:, fbo[c] : fbo[c] + CHUNKS[c] // 32],
                bias=1.0, scale=-1.0,
            ).then_inc(act_r1, 1)

    return nc


def kernel(anchors: np.ndarray, gt_boxes: np.ndarray) -> np.ndarray:
    global LAST_EXEC_NS
    anchors = np.asarray(anchors, dtype=np.float32) * COORD_SCALE
    gt_boxes = np.asarray(gt_boxes, dtype=np.float32) * COORD_SCALE

    apad = np.zeros((N_PAD, 4), dtype=np.float32)
    apad[:N_ANCHORS] = anchors

    # gt scalar columns: partition p = b*32 + gh; g = gh*2 + gl
    g = gt_boxes.reshape(BATCH, N_GT, 4)
    gcols = np.zeros((P, 14), dtype=np.float32)
    bs = np.repeat(np.arange(BATCH), 32)
    gh = np.tile(np.arange(32), BATCH)
    for gl in range(2):
        gg = g[bs, gh * 2 + gl]          # [128, 4] (x1,y1,x2,y2)
        gcols[:, 4 * gl + 0] = gg[:, 0]  # gx1
        gcols[:, 4 * gl + 1] = gg[:, 2]  # gx2
        gcols[:, 4 * gl + 2] = gg[:, 1]  # gy1
        gcols[:, 4 * gl + 3] = gg[:, 3]  # gy2
        gcols[:, 8 + gl] = (gg[:, 2] - gg[:, 0]) * (gg[:, 3] - gg[:, 1])
        gcols[:, 10 + gl] = -gg[:, 1]    # -gy1 (relu-form bias)
        gcols[:, 12 + gl] = gg[:, 3] - gg[:, 1]  # gheight

    in_maps = []
    for c in range(N_CORES):
        sh = apad[c * N_LOC : (c + 1) * N_LOC]
        arows = np.empty((5, N_LOC), dtype=np.float16)
        arows[0] = sh[:, 0]  # ax1
        arows[1] = sh[:, 2]  # ax2
        arows[2] = sh[:, 1]  # ay1
        arows[3] = sh[:, 3]  # ay2
        arows[4] = (sh[:, 2] - sh[:, 0]) * (sh[:, 3] - sh[:, 1])
        in_maps.append({"arows": np.ascontiguousarray(arows), "gcols": gcols})

    nc = _build_graph()
    trace = os.environ.get("ANCHOR_TRACE", "0") == "1"
    core_ids = list(range(N_CORES))
    if trace:
        _ensure_axon_ntff_hook()
        _patch_upload_artifacts()
        try:
            res = run_bass_kernel_spmd(nc, in_maps, core_ids=core_ids, trace=True)
        except Exception as e:
            print(
                f"trace run failed ({type(e).__name__}: {e}); falling back",
                file=sys.stderr,
            )
            res = run_bass_kernel_spmd(nc, in_maps, core_ids=core_ids, trace=False)
    else:
        res = run_bass_kernel_spmd(nc, in_maps, core_ids=core_ids, trace=False)
    LAST_EXEC_NS = res.exec_time_ns

    out = np.empty((BATCH, N_PAD), dtype=np.float32)
    for c in range(N_CORES):
        o = res.results[c]["out"].reshape(BATCH, 32, N_LOC // 32)
        # value(b, i, fb) is anchor a = fb*32 + i
        out[:, c * N_LOC : (c + 1) * N_LOC] = o.transpose(0, 2, 1).reshape(
            BATCH, N_LOC
        )
    return out[:, :N_ANCHORS]


# revision 28
# speedup vs baseline: 1.0219x; 1.0219x over previous
"""AnchorTargetLayer max-IoU kernel for 8 TRN2 NeuronCores (v7.3).

max_iou[b, n] = max_g IoU(anchor_n, gt_box[b, g]);
anchors [100000, 4] f32, gt_boxes [4, 64, 4] f32 -> out [4, 100000] f32.

Sharding: anchors split 8 ways (12544/core incl pad), gt replicated, no
collectives.  HW exec ~132-134us vs the v5 baseline's 167.6us.

Layout (v7, vs v5's anchors-on-partitions): (batch, gt) pairs live on
PARTITIONS - p = b*32 + gh, gl in {0,1} on the free dim (g = gh*2+gl) -
and ANCHORS live on the FREE dim, broadcast to all 128 partitions by a
partition-stride-0 DMA (fully overlapped, ~380 GB/s).  This replaces
v5's 392 narrow 256-wide tensor_scalar ops (~360ns each, fixed-overhead
bound) with a handful of wide ops per chunk.

Chunks of <=3136 anchors ([1024, 3136, 2112, 3136, 2112, 1024]: small
head chunk starts the pipeline early; the first y-relu chunk (2) is
small and follows a big clip-form chunk so the ACT engine's relus for
it land well before DVE needs them (otherwise a DMA-timing race makes
runs bimodal 132/143us); small tail chunk shortens the drain; chunk 0's
broadcast is split x-rows-first so clips-x start ~3us earlier).  Per chunk, all fp16:
  x spans:  CHL[gl] = clamp(ax1/ax2 rows; gx1[p], gx2[p])  2 TS 2-op @4x
            Ix = CHL[hi] - CHL[lo]  (>= 0 by clipping)     1 wide TT
  y spans:  chunks 0-1 (clip form, no ACT dependency at pipeline head):
              same as x on DVE;
            chunks 2+ (relu form, offloaded to the idle ACT engine):
              ACT: r1y = relu(-ay2 + gy2[p]), r2y = relu(ay1 - gy1[p])
              DVE: SY = (r1y + r2y)*(-1) + gheight[p]   (TT + TS @4x)
  int:      INT = Ix * IY                                  1 TT
  w:        W = INT * RS;  RS = 1/(aarea + garea[p]) on ACT (Reciprocal
            with per-partition bias: no separate area-sum pass)
  gl max:   R0 = max(W[gl0], W[gl1])                       1 TT
  gh max:   stream_transpose 32x32 (b-major partitions put gh innermost
            per anchor subblock), then a 5-step pairwise TT-max tree.
Fixups per chunk (overlapped): ACT R1 = 1/(1 - v); DVE STT
MIOU = max(v, 0) * R1  (w = int/(areaA+areaG) is monotone in iou, so
iou = v/(1-v) after the max; the max(.,0) guards relu-form negatives).
Per-chunk output DMA drains while later chunks compute.

Notes: w <= 1/2 so 1-v >= 1/2 (no cancellation); scalar_tensor_tensor
runs at 1x but only on the tiny [P, fc/32] fixup.  SBUF: the CHL slot
block is reused for SY/INT/W/R0/ST/tree scratch (chain is serial on
DVE); ABC, RS, RRY are double-buffered across chunks.  Engine balance:
DVE ~110us busy, ACT ~60us, head ~18us is DMA-latency bound.

Learned constraints (measured/compile-tested this session): TT needs
equal base partitions (no partition-offset tree); TT reads at most one
PSUM operand; compute ops cannot write at a partition offset; custom
DVE ops and gpsimd partition_all_reduce fail walrus codegen ("ISA wrong
length"); Pool TT is rejected ("engine check failed"); stream_transpose
free dims of 800 corrupt downstream 128-col groups (1024/1568/2112/3136
are clean); DMA-transpose tiles serialize at ~1.24us/tile on one queue.
"""

import os
import sys

import numpy as np

sys.path.insert(0, "/opt/trn_rl_repo")

import concourse.bass as bass
import concourse.mybir as mybir
from concourse import masks
from concourse.bass_utils import run_bass_kernel_spmd

N_ANCHORS = 100000
BATCH = 4
N_GT = 64
N_CORES = 8

P = 128
FC = 3136                   # max anchors per chunk
# small head chunks for fast pipeline start; small tail chunk shortens drain
CHUNKS = [1024, 3136, 2112, 3136, 2112, 1024]
NCH = len(CHUNKS)
N_LOC = sum(CHUNKS)         # 12544
N_PAD = N_LOC * N_CORES     # 100352
NB = 2                      # ABC / RS double-buffer depth

F32 = mybir.dt.float32
F16 = mybir.dt.float16
COORD_SCALE = 1.0 / 16.0

LAST_EXEC_NS = None


def _ensure_axon_ntff_hook():
    try:
        import antenv.axon_hooks  # noqa: F401

        return
    except ImportError:
        pass
    import contextlib
    import ctypes
    import types

    import antenv

    m = types.ModuleType("antenv.axon_hooks")
    m._hook = None

    def set_axon_ntff_profile_hook(h):
        m._hook = h

    def get_axon_ntff_profile_hook():
        return m._hook

    m.set_axon_ntff_profile_hook = set_axon_ntff_profile_hook
    m.get_axon_ntff_profile_hook = get_axon_ntff_profile_hook
    sys.modules["antenv.axon_hooks"] = m
    antenv.axon_hooks = m

    so_path = os.environ.get("PJRT_LIBRARY_PATH", "/opt/axon/libaxon_pjrt.so")
    try:
        lib = ctypes.CDLL(so_path)
    except OSError:
        return
    if not hasattr(lib, "axon_start_nrt_profile"):
        return
    lib.axon_start_nrt_profile.argtypes = [
        ctypes.POINTER(ctypes.c_int64),
        ctypes.c_size_t,
    ]
    lib.axon_start_nrt_profile.restype = ctypes.c_int64
    lib.axon_stop_nrt_profile.argtypes = [ctypes.c_char_p]
    lib.axon_stop_nrt_profile.restype = ctypes.c_int64

    @contextlib.contextmanager
    def _hook(output_dir, device_ids):
        import jax

        jax.devices()
        if device_ids:
            ids = (ctypes.c_int64 * len(device_ids))(*device_ids)
            rc = lib.axon_start_nrt_profile(ids, len(device_ids))
        else:
            rc = lib.axon_start_nrt_profile(None, 0)
        if rc != 0:
            raise RuntimeError(f"axon_start_nrt_profile rc={rc}")
        try:
            yield
        finally:
            n = lib.axon_stop_nrt_profile(str(output_dir).encode())
            if n < 0:
                raise RuntimeError(f"axon_stop_nrt_profile rc={n}")

    set_axon_ntff_profile_hook(_hook)


def _patch_upload_artifacts():
    import concourse.bass_utils as bu

    if getattr(bu.upload_artifacts, "_safe", False):
        return
    orig = bu.upload_artifacts

    def safe(tmpdir):
        try:
            return orig(tmpdir)
        except Exception:
            return tmpdir

    safe._safe = True
    bu.upload_artifacts = safe


def _act_recip(scalar_eng, nc, out_ap, in_ap, bias=0.0, scale=1.0):
    """Emit Activation(Reciprocal); bias may be a [P,1] AP or a float.
    (The nc.scalar.activation wrapper rejects Reciprocal.)"""
    ins = [scalar_eng.lower_ap(in_ap)]
    if isinstance(bias, (int, float)):
        ins.append(mybir.ImmediateValue(dtype=F32, value=float(bias)))
    else:
        ins.append(scalar_eng.lower_ap(bias))
    ins.append(mybir.ImmediateValue(dtype=F32, value=float(scale)))
    ins.append(mybir.ImmediateValue(dtype=F32, value=0.0))
    return scalar_eng.add_instruction(
        mybir.InstActivation(
            name=nc.get_next_instruction_name(),
            func=mybir.ActivationFunctionType.Reciprocal,
            ins=ins,
            outs=[scalar_eng.lower_ap(out_ap)],
        )
    )


def _build_graph():
    nc = bass.Bass()
    AR_ext = nc.declare_dram_parameter("arows", [5, N_LOC], F16, isOutput=False)
    GC_ext = nc.declare_dram_parameter("gcols", [P, 14], F32, isOutput=False)
    NFB = N_LOC // 32
    NVO_ = 4 * sum((fc + 127) // 128 for fc in CHUNKS)
    out_ext = nc.declare_dram_parameter("out", [P, NVO_], F32, isOutput=True)

    Alu = mybir.AluOpType
    from contextlib import ExitStack

    with ExitStack() as _st:
        e = _st.enter_context

        ABC = e(nc.sbuf_tensor("ABC", [P, NB, 5, FC], F16))
        GCS = e(nc.sbuf_tensor("GCS", [P, 14], F32))
        # CHL: clip outputs [ax, gl, endp(lo,hi), FC]; its 8*FC region is
        # reused downstream: INT <- CHL[0,0], W <- CHL[0,1], R0/ST <- CHL[1,0],
        # tree scratch <- CHL[1,1] (the whole chain is serial on DVE).
        CHL = e(nc.sbuf_tensor("CHL", [P, 2, 2, 2, FC], F16))
        I = e(nc.sbuf_tensor("I", [P, 2, FC], F16))
        RRY = e(nc.sbuf_tensor("RRY", [P, NB, 2, 2, FC], F16))
        RS = e(nc.sbuf_tensor("RS", [P, NB, 2, FC], F16))
        NVO = 4 * sum((fc + 127) // 128 for fc in CHUNKS)
        VOUT = e(nc.sbuf_tensor("VOUT", [P, NVO], F16))
        R1 = e(nc.sbuf_tensor("R1", [P, NVO], F16))
        MIOU = e(nc.sbuf_tensor("MIOU", [P, NVO], F32))
        IDT = e(nc.sbuf_tensor("IDT", [P, 128], F16))
        PS = e(nc.psum_tensor("PS", [P, (FC + 127) // 128, 128], F32))

        block = e(nc.Block())
        dma_sem = e(nc.semaphore("dma_sem"))
        dma_act = e(nc.semaphore("dma_act"))
        dve_sem = e(nc.semaphore("dve_sem"))
        miou_sem = e(nc.semaphore("miou_sem"))
        act_rs = e(nc.semaphore("act_rs"))
        act_r1 = e(nc.semaphore("act_r1"))
        r0_sem = e(nc.semaphore("r0_sem"))
        pe_sem = e(nc.semaphore("pe_sem"))
        rd_sem = e(nc.semaphore("rd_sem"))
        idt_sem = e(nc.semaphore("idt_sem"))

        def gcol(i):
            return GCS[:, i : i + 1]

        offs = [sum(CHUNKS[:i]) for i in range(NCH)]
        fbo = [o // 32 for o in offs]
        tiles = [(fc + 127) // 128 for fc in CHUNKS]
        vo = [4 * sum(tiles[:i]) for i in range(NCH + 1)]  # VOUT col offsets

        # dve_sem ticks: 2c+1 = clips(c) done, 2c+2 = tree(c) done.
        # x-rows (0:2) broadcast on the SP queue (consumed by DVE clips);
        # y+area rows (2:5) broadcast in parallel on the ACT queue
        # (consumed by the ACT reciprocals; y rows are spare for now).
        def bcast(eng, c, rows, sem):
            fc = CHUNKS[c]
            a_ap = AR_ext[rows[0] : rows[1], offs[c] : offs[c] + fc]
            a_b = bass.AP(
                tensor=a_ap.tensor, offset=a_ap.offset,
                ap=[[0, P]] + list(a_ap.ap),
            )
            return eng.dma_start(
                out=ABC[:, c % NB, rows[0] : rows[1], 0:fc], in_=a_b
            ).then_inc(sem, 16)

        @block.sync
        def _(sync):
            sync.dma_start(out=GCS[:, :], in_=GC_ext[:, :]).then_inc(dma_sem, 16)
            for c in range(NCH):
                if c >= NB:
                    sync.wait_ge(dve_sem, c - 1)
                    sync.wait_ge(act_rs, c - 1)
                if c == 0:
                    # split: x rows land first so clips-x(0) starts early
                    bcast(sync, 0, (0, 2), dma_sem)
                    bcast(sync, 0, (2, 5), dma_sem)
                else:
                    bcast(sync, c, (0, 5), dma_sem)
            # per-chunk output drains as soon as each MIOU slab is written
            for c in range(NCH):
                sync.wait_ge(miou_sem, c + 1)
                sync.dma_start(
                    out=out_ext[:, vo[c] : vo[c + 1]],
                    in_=MIOU[:, vo[c] : vo[c + 1]],
                ).then_inc(dma_sem, 16)
            sync.wait_ge(dma_sem, 16 * (2 * NCH + 2))

        @block.gpsimd
        def _(gpsimd):
            masks.make_identity(nc, IDT[:, :])
            gpsimd.memset(IDT[0:1, 0:1], 1.0).then_inc(idt_sem, 1)

        @block.tensor
        def _(tensor):
            tensor.wait_ge(idt_sem, 1)
            for c in range(NCH):
                fc = CHUNKS[c]
                yr = c >= 2
                r0_src = CHL[:, 1, 0, 1, :] if not yr else CHL[:, 1, 0, 0, :]
                tensor.wait_ge(r0_sem, c + 1)
                if c >= 1:
                    tensor.wait_ge(rd_sem, c)  # PSUM free (reduce(c-1) done)
                for t in range(tiles[c]):
                    m = min(128, fc - 128 * t)
                    ins = tensor.matmul(
                        out=PS[0:m, t, :],
                        lhsT=r0_src[:, 128 * t : 128 * t + m],
                        rhs=IDT[:, :],
                        start=True, stop=True,
                    )
                ins.then_inc(pe_sem, 1)

        @block.vector
        def _(vector):
            for c in range(NCH):
                cb = c % NB
                fc = CHUNKS[c]
                fb = fc // 32
                # 4 clip TS 2-op @4x: CHL[ax,gl] = (a_rows max g1) min g2
                # x rows arrive on the SP queue, y rows on the ACT queue
                vector.wait_ge(dma_sem, 16 * (2 + c) if c == 0 else 16 * (3 + c))
                yrelu = c >= 2
                # x clips always; y clips only for head (clip-form) chunks
                for ax in range(2 if not yrelu else 1):
                    if c == 0 and ax == 1:
                        vector.wait_ge(dma_sem, 48)  # y/area rows of chunk 0
                    rows = ABC[:, cb, 2 * ax : 2 * ax + 2, 0:fc]  # [P, 2, fc]
                    for gl in range(2):
                        clip_ins = vector.tensor_scalar(
                            out=CHL[:, ax, gl, :, 0:fc], in0=rows,
                            scalar1=gcol(4 * gl + 2 * ax + 0),
                            scalar2=gcol(4 * gl + 2 * ax + 1),
                            op0=Alu.max, op1=Alu.min,
                        )
                clip_ins.then_inc(dve_sem, 1)  # tick c+1
                # reduce of the previous chunk: PSUM -> VOUT slab
                if c >= 1:
                    pc = c - 1
                    vector.wait_ge(pe_sem, c)
                    vector.tensor_reduce(
                        out=VOUT[:, vo[pc] : vo[pc + 1]].rearrange(
                            "p (t b) -> p t b", b=4
                        ),
                        in_=PS[:, 0 : tiles[pc], :].rearrange(
                            "p t (b g) -> p t b g", g=32
                        ),
                        axis=mybir.AxisListType.X, op=Alu.max,
                    ).then_inc(rd_sem, 1)
                # Ix = hi - lo  [P, 2gl, fc]
                vector.tensor_tensor(
                    out=I[:, :, 0:fc],
                    in0=CHL[:, 0, :, 1, 0:fc],
                    in1=CHL[:, 0, :, 0, 0:fc],
                    op=Alu.subtract,
                )
                if yrelu:
                    # SY = gh - r1y - r2y  (ACT supplied RRY)
                    vector.wait_ge(act_rs, c + 1)
                    vector.tensor_tensor(
                        out=CHL[:, 1, :, 0, 0:fc],
                        in0=RRY[:, cb, :, 0, 0:fc], in1=RRY[:, cb, :, 1, 0:fc],
                        op=Alu.add,
                    )
                    for gl in range(2):
                        vector.tensor_scalar(
                            out=CHL[:, 1, gl, 1, 0:fc],
                            in0=CHL[:, 1, gl, 0, 0:fc],
                            scalar1=-1.0, scalar2=gcol(12 + gl),
                            op0=Alu.mult, op1=Alu.add,
                        )
                    iy = CHL[:, 1, :, 1, 0:fc]
                else:
                    # Iy via clip-form: hi - lo from the y clips
                    vector.tensor_tensor(
                        out=CHL[:, 0, :, 0, 0:fc],
                        in0=CHL[:, 1, :, 1, 0:fc],
                        in1=CHL[:, 1, :, 0, 0:fc],
                        op=Alu.subtract,
                    )
                    iy = CHL[:, 0, :, 0, 0:fc]
                # INT = Ix * Iy  [P, 2gl, fc]
                int_out = CHL[:, 0, :, 1, 0:fc] if not yrelu else CHL[:, 0, :, 0, 0:fc]
                vector.tensor_tensor(
                    out=int_out, in0=I[:, :, 0:fc], in1=iy, op=Alu.mult,
                )
                # W = INT * RS
                if not yrelu:
                    vector.wait_ge(act_rs, c + 1)
                w_out = CHL[:, 1, :, 0, 0:fc] if not yrelu else CHL[:, 0, :, 1, 0:fc]
                vector.tensor_tensor(
                    out=w_out, in0=int_out, in1=RS[:, cb, :, 0:fc], op=Alu.mult,
                )
                # fixup of the previous chunk (ACT computed R1 from reduce)
                if c >= 1:
                    vector.wait_ge(act_r1, c)
                    pc = c - 1
                    vector.scalar_tensor_tensor(
                        out=MIOU[:, vo[pc] : vo[pc + 1]],
                        in0=VOUT[:, vo[pc] : vo[pc + 1]],
                        scalar=0.0,
                        in1=R1[:, vo[pc] : vo[pc + 1]],
                        op0=Alu.max, op1=Alu.mult,
                    ).then_inc(miou_sem, 1)
                # R0 = max over gl; PE transposes it into PSUM
                r0_out = CHL[:, 1, 0, 1, 0:fc] if not yrelu else CHL[:, 1, 0, 0, 0:fc]
                vector.tensor_tensor(
                    out=r0_out, in0=w_out[:, 0, :], in1=w_out[:, 1, :],
                    op=Alu.max,
                ).then_inc(r0_sem, 1)
            # last chunk: reduce then fixup
            c = NCH - 1
            vector.wait_ge(pe_sem, NCH)
            vector.tensor_reduce(
                out=VOUT[:, vo[c] : vo[c + 1]].rearrange("p (t b) -> p t b", b=4),
                in_=PS[:, 0 : tiles[c], :].rearrange("p t (b g) -> p t b g", g=32),
                axis=mybir.AxisListType.X, op=Alu.max,
            ).then_inc(rd_sem, 1)
            vector.wait_ge(act_r1, NCH)
            vector.scalar_tensor_tensor(
                out=MIOU[:, vo[c] : vo[c + 1]],
                in0=VOUT[:, vo[c] : vo[c + 1]],
                scalar=0.0,
                in1=R1[:, vo[c] : vo[c + 1]],
                op0=Alu.max, op1=Alu.mult,
            ).then_inc(miou_sem, 1)

        @block.scalar
        def _(scalar):
            for c in range(NCH):
                cb = c % NB
                fc = CHUNKS[c]
                scalar.wait_ge(dma_sem, 48 if c == 0 else 16 * (3 + c))
                if c >= 2:
                    if c >= 4:
                        # RRY[cb] reuse: r12y(c-2) done once clips(c-1) ran
                        scalar.wait_ge(dve_sem, c)
                    # y-relu form: r1y = relu(-y2 + gy2), r2y = relu(y1 - gy1)
                    for gl in range(2):
                        scalar.activation(
                            out=RRY[:, cb, gl, 0, 0:fc], in_=ABC[:, cb, 3, 0:fc],
                            func=mybir.ActivationFunctionType.Relu,
                            bias=gcol(4 * gl + 3), scale=-1.0,
                        )
                        scalar.activation(
                            out=RRY[:, cb, gl, 1, 0:fc], in_=ABC[:, cb, 2, 0:fc],
                            func=mybir.ActivationFunctionType.Relu,
                            bias=gcol(10 + gl), scale=1.0,
                        )
                if c >= NB:
                    # W(c-2) has read RS[cb]: implied by clips(c-1) done
                    scalar.wait_ge(dve_sem, c)
                area = ABC[:, cb, 4, 0:fc]
                _act_recip(scalar, nc, RS[:, cb, 0, 0:fc], area, bias=gcol(8))
                _act_recip(
                    scalar, nc, RS[:, cb, 1, 0:fc], area, bias=gcol(9)
                ).then_inc(act_rs, 1)
                # R1 fixup for an earlier finished chunk: interleave to avoid
                # stalling the recips; chunk c-1's tree is done by now or soon
                if c >= 1:
                    pc = c - 1
                    scalar.wait_ge(rd_sem, pc + 1)
                    _act_recip(
                        scalar, nc,
                        R1[:, vo[pc] : vo[pc + 1]],
                        VOUT[:, vo[pc] : vo[pc + 1]],
                        bias=1.0, scale=-1.0,
                    ).then_inc(act_r1, 1)
            c = NCH - 1
            scalar.wait_ge(rd_sem, c + 1)
            _act_recip(
                scalar, nc, R1[:, vo[c] : vo[c + 1]],
                VOUT[:, vo[c] : vo[c + 1]],
                bias=1.0, scale=-1.0,
            ).then_inc(act_r1, 1)

    return nc


def kernel(anchors: np.ndarray, gt_boxes: np.ndarray) -> np.ndarray:
    global LAST_EXEC_NS
    anchors = np.asarray(anchors, dtype=np.float32) * COORD_SCALE
    gt_boxes = np.asarray(gt_boxes, dtype=np.float32) * COORD_SCALE

    apad = np.zeros((N_PAD, 4), dtype=np.float32)
    apad[:N_ANCHORS] = anchors

    # gt scalar columns: partition p = b*32 + gh; g = gh*2 + gl
    g = gt_boxes.reshape(BATCH, N_GT, 4)
    gcols = np.zeros((P, 14), dtype=np.float32)
    bs = np.repeat(np.arange(BATCH), 32)
    gh = np.tile(np.arange(32), BATCH)
    for gl in range(2):
        gg = g[bs, gh * 2 + gl]          # [128, 4] (x1,y1,x2,y2)
        gcols[:, 4 * gl + 0] = gg[:, 0]  # gx1
        gcols[:, 4 * gl + 1] = gg[:, 2]  # gx2
        gcols[:, 4 * gl + 2] = gg[:, 1]  # gy1
        gcols[:, 4 * gl + 3] = gg[:, 3]  # gy2
        gcols[:, 8 + gl] = (gg[:, 2] - gg[:, 0]) * (gg[:, 3] - gg[:, 1])
        gcols[:, 10 + gl] = -gg[:, 1]    # -gy1 (relu-form bias)
        gcols[:, 12 + gl] = gg[:, 3] - gg[:, 1]  # gheight

    in_maps = []
    for c in range(N_CORES):
        sh = apad[c * N_LOC : (c + 1) * N_LOC]
        arows = np.empty((5, N_LOC), dtype=np.float16)
        arows[0] = sh[:, 0]  # ax1
        arows[1] = sh[:, 2]  # ax2
        arows[2] = sh[:, 1]  # ay1
        arows[3] = sh[:, 3]  # ay2
        arows[4] = (sh[:, 2] - sh[:, 0]) * (sh[:, 3] - sh[:, 1])
        in_maps.append({"arows": np.ascontiguousarray(arows), "gcols": gcols})

    nc = _build_graph()
    trace = os.environ.get("ANCHOR_TRACE", "0") == "1"
    core_ids = list(range(N_CORES))
    if trace:
        _ensure_axon_ntff_hook()
        _patch_upload_artifacts()
        try:
            res = run_bass_kernel_spmd(nc, in_maps, core_ids=core_ids, trace=True)
        except Exception as e:
            print(
                f"trace run failed ({type(e).__name__}: {e}); falling back",
                file=sys.stderr,
            )
            res = run_bass_kernel_spmd(nc, in_maps, core_ids=core_ids, trace=False)
    else:
        res = run_bass_kernel_spmd(nc, in_maps, core_ids=core_ids, trace=False)
    LAST_EXEC_NS = res.exec_time_ns

    out = np.empty((BATCH, N_PAD), dtype=np.float32)
    tiles = [(fc + 127) // 128 for fc in CHUNKS]
    for c in range(N_CORES):
        o = res.results[c]["out"]  # [128, 4*sum(tiles)]: cols (tile, b)
        cum = 0
        off = 0
        for ci, fc in enumerate(CHUNKS):
            nt = tiles[ci]
            blk = o[:, 4 * cum : 4 * (cum + nt)].reshape(P, nt, 4)
            # value[p, t, b] is anchor offs c*N_LOC + off + t*128 + p
            arr = blk.transpose(2, 1, 0).reshape(BATCH, nt * 128)[:, :fc]
            out[:, c * N_LOC + off : c * N_LOC + off + fc] = arr
            cum += nt
            off += fc
    return out[:, :N_ANCHORS]
